# revision 58
# baseline (speedup 1.0000x reference)
"""Trainium2 Bass kernel for one pre-LN transformer block (B=4, T=1024, C=1024,
H=16 heads, FF=4096), distributed over 8 NeuronCores with no collectives.

Sharding: core = (batch b, query-parity j). Each core computes K/V for all 1024
tokens of its batch but attention/FFN only for its 512 queries (tokens t with
t % 2 == j). Interleaved queries make the causal-mask tile structure identical
on every core (SPMD-safe) while skipping ~37.5% of score/AV work. The host
only permutes/transposes inputs and re-interleaves the outputs.

On-device layout: activations live transposed [feature, token] so the whole
chain (LN1 -> QKV -> scores -> AV -> proj -> LN2 -> FFN) is matmul-native.
The big GEMMs (QKV, proj, FFN1, FFN2, AV) run in fp8e4 with the DoubleRow
perf mode (256-deep contraction per pass, 2x PE throughput); weights are
host-scaled by 64 for fp8 range and the 1/64 is folded into downstream
scale/bias ops (exp scale, gelu scale, softmax-denominator ones column,
residual-add scalar). Scores run fp8 single-row (K=64). LayerNorm stats are
computed with ones-vector matmuls on a bf16 cast; accumulation is f32 in
PSUM and both residual streams are carried in f32.
"""

import math
import sys
from dataclasses import dataclass

if "/opt/trn_rl_repo" not in sys.path:
    sys.path.insert(0, "/opt/trn_rl_repo")

import numpy as np

WS = 64.0  # host-side weight scale for fp8


@dataclass(frozen=True)
class Cfg:
    B: int = 4
    T: int = 1024
    C: int = 1024
    H: int = 16
    FF: int = 4096

    @property
    def HD(self):
        return self.C // self.H

    @property
    def TQ(self):  # queries per core
        return self.T // 2

    @property
    def NCI(self):  # C / 128 feature tiles
        return self.C // 128

    @property
    def NFF(self):  # FF / 128 hidden tiles
        return self.FF // 128

    @property
    def NKB(self):  # key blocks of 128
        return self.T // 128

    @property
    def NKP(self):  # key blocks per parity half
        return self.NKB // 2

    @property
    def BW(self):  # token block width for LN1 / K phases
        return min(512, self.T)

    @property
    def NTB(self):  # token blocks over all T tokens
        return self.T // self.BW

    @property
    def NQB(self):  # 512-col blocks over TQ (==1 at full size)
        return self.TQ // 512 if self.TQ >= 512 else 1

    @property
    def QW(self):  # query block width
        return min(self.TQ, 512)

    def s_kb(self, kb: int) -> int:
        """Start query-column of the computed score region for key block kb.
        Key blocks 0..NKP-1 hold this core's own-parity tokens in order;
        NKP.. hold the complementary-parity tokens. Causality allows a
        suffix of queries per block; the two parity halves share s per kbp."""
        return min(128 * (kb % self.NKP), self.TQ)

    def mask_w(self, kb: int) -> int:
        """Width of the region of score columns that needs the multiplicative
        mask (the partially-visible diagonal zone)."""
        kbp = kb % self.NKP
        end = min(128 * kbp + 128, self.TQ)
        return end - self.s_kb(kb)

    @property
    def MW(self):
        return max(self.mask_w(kb) for kb in range(self.NKB))

    @property
    def kb_order(self):
        """Packing order of the kbp score regions inside a parity half.
        [0,1,3,2] puts the masked 128-col zones of regions 3 and 2 adjacent
        (cols 896..1152) so each head needs only two batched mask multiplies:
        zones of kbp 0,1 sit at cols 0 and 512 (stride 512)."""
        return [0, 1, 3, 2]

    @property
    def pt_offs(self):
        """Column offset of each kbp's packed score region within a parity
        half (kb_order layout), plus the total width."""
        off = {}
        o = 0
        for kbp in self.kb_order:
            off[kbp] = o
            o += self.TQ - self.s_kb(kbp)
        return off, o


def build_nc(cfg: Cfg, n_cores: int = 8):
    import concourse.bass as bass
    import concourse.tile as tile
    from concourse import bacc, mybir

    f32 = mybir.dt.float32
    f32r = mybir.dt.float32r
    bf16 = mybir.dt.bfloat16
    f8 = mybir.dt.float8e4
    Act = mybir.ActivationFunctionType
    Alu = mybir.AluOpType
    DR = mybir.MatmulPerfMode.DoubleRow

    C, H, HD, FF = cfg.C, cfg.H, cfg.HD, cfg.FF
    NCI, NFF, NKB, NTB = cfg.NCI, cfg.NFF, cfg.NKB, cfg.NTB
    NKP = cfg.NKP
    TQ, QW, T = cfg.TQ, cfg.QW, cfg.T
    NC2 = NCI // 2  # DR contraction steps over C
    NF2 = NFF // 2  # DR contraction steps over FF
    scale = 1.0 / math.sqrt(HD)
    inv_ws2 = 1.0 / (WS * WS)

    nc = bacc.Bacc(
        "TRN2", target_bir_lowering=False, debug=False, num_devices=n_cores
    )

    # ---- DRAM I/O ----
    xpt = nc.dram_tensor("xpt", [C, T], bf16, kind="ExternalInput")
    msk = nc.dram_tensor("msk", [NKB, 128, cfg.MW], f8, kind="ExternalInput")
    ident = nc.dram_tensor("ident", [HD, 2, 128], f8, kind="ExternalInput")
    # DR-packed, WS-scaled fp8 weights: [k2, p, j, f] = WS * W[k2*256+j*128+p, f]
    wq = nc.dram_tensor("wq", [NC2, 128, 2, C], f8, kind="ExternalInput")
    wk = nc.dram_tensor("wk", [NC2, 128, 2, C], f8, kind="ExternalInput")
    wv = nc.dram_tensor("wv", [NC2, 128, 2, C], f8, kind="ExternalInput")
    wp = nc.dram_tensor("wp", [NC2, 128, 2, C], f8, kind="ExternalInput")
    w1 = nc.dram_tensor("w1", [C, FF], bf16, kind="ExternalInput")
    w2 = nc.dram_tensor("w2", [FF, C], bf16, kind="ExternalInput")
    ln1g = nc.dram_tensor("ln1g", [C], f32r, kind="ExternalInput")
    ln1b = nc.dram_tensor("ln1b", [C], f32r, kind="ExternalInput")
    ln2g = nc.dram_tensor("ln2g", [C], f32r, kind="ExternalInput")
    ln2b = nc.dram_tensor("ln2b", [C], f32r, kind="ExternalInput")
    bpj = nc.dram_tensor("bpj", [C], f32, kind="ExternalInput")
    b1 = nc.dram_tensor("b1", [FF], f32, kind="ExternalInput")
    b2 = nc.dram_tensor("b2", [C], f32, kind="ExternalInput")
    yt = nc.dram_tensor("yt", [C, TQ], f32, kind="ExternalOutput")

    with (
        nc.allow_low_precision(reason="fp8/bf16 rounding of matmul operands"),
        tile.TileContext(nc) as tc,
    ):
        # ---------------- persistent constants / params ----------------
        onesf, free_onesf = tc.tile([128, 128], f32, name="onesf")
        nc.vector.memset(onesf, 1.0)
        ws_f, free_ws_f = tc.tile([128, H], f32, name="ws_f")
        nc.vector.memset(ws_f, WS)
        inv64, free_inv64 = tc.tile([128, 1], f32, name="inv64")
        nc.vector.memset(inv64, 1.0 / WS)
        ones128, free_ones128 = tc.tile([128, 1], bf16, name="ones128")
        nc.vector.tensor_copy(out=ones128, in_=onesf[:, 0:1])
        ones_row, free_ones_row = tc.tile([1, 128], bf16, name="ones_row")
        nc.vector.tensor_copy(out=ones_row, in_=onesf[0:1, 0:128])
        # lhsT row of ones at partition 64 for the per-head recip broadcast
        oneshi, free_oneshi = tc.tile([65, HD], bf16, name="oneshi")
        nc.vector.tensor_copy(out=oneshi, in_=onesf[0:65, 0:HD])
        epst, free_epst = tc.tile([1, 1], f32, name="epst")
        nc.vector.memset(epst, 1e-5)
        identsb, free_identsb = tc.tile([HD, 2, 128], f8, name="identsb")
        nc.sync.dma_start(out=identsb, in_=ident[:, :, :])

        ln1gp, free_ln1gp = tc.tile([128, NCI], f32, name="ln1gp")
        ln1bp, free_ln1bp = tc.tile([128, NCI], f32, name="ln1bp")
        ln2gp, free_ln2gp = tc.tile([128, NCI], f32, name="ln2gp")
        ln2bp, free_ln2bp = tc.tile([128, NCI], f32, name="ln2bp")
        for ptile, v in (
            (ln1gp, ln1g), (ln1bp, ln1b), (ln2gp, ln2g), (ln2bp, ln2b)
        ):
            nc.sync.dma_start(
                out=ptile, in_=v.rearrange("(a p) -> p a", p=128).bitcast(f32)
            )
        bpjt, free_bpjt = tc.tile([128, NCI], f32, name="bpjt")
        nc.sync.dma_start(out=bpjt, in_=bpj.rearrange("(a p) -> p a", p=128))
        b1t, free_b1t = tc.tile([128, NFF], f32, name="b1t")
        nc.sync.dma_start(out=b1t, in_=b1.rearrange("(a p) -> p a", p=128))
        b2t, free_b2t = tc.tile([128, NCI], f32, name="b2t")
        nc.sync.dma_start(out=b2t, in_=b2.rearrange("(a p) -> p a", p=128))
        # one PSUM pool + one weight-stream pool for the whole kernel:
        # per-phase pools would serialize phases at their alloc/release
        # boundaries (a pool alloc waits on the previous pool's release,
        # which waits on its last reader).
        ps_all = tc.alloc_tile_pool(name="ps_all", bufs=8, space="PSUM")
        wstream = tc.alloc_tile_pool(name="wstream", bufs=8)

        # x2T = x + attnproj + bproj (residual 1), written in the proj phase
        x2t, free_x2t = tc.tile([128, NCI, TQ], f32r, name="x2t")
        # residual-1 operand: x (query columns) + bproj, set in the QKV phase
        rawq, free_rawq = tc.tile([128, NCI, TQ], bf16, name="rawq")

        mskt, free_mskt = tc.tile([128, NKB, cfg.MW], f8, name="mskt")

        raw, free_raw = tc.tile([128, NCI, T], bf16, name="raw")
        xpt_r = xpt.rearrange("(ci p) t -> ci p t", p=128)
        for tb in range(NTB):  # block-0 columns first: LN1 starts sooner
            sl = slice(tb * cfg.BW, (tb + 1) * cfg.BW)
            for ci in range(NCI):
                nc.sync.dma_start(out=raw[:, ci, sl], in_=xpt_r[ci][:, sl])
        nc.sync.dma_start(out=mskt, in_=msk.rearrange("k p m -> p k m"))

        def layernorm(src_ap_fn, dst, gp, bp, n_blocks, blk_w, scopename,
                      cast_fn=None):
            """src_ap_fn(ci, sl) -> [128, blk_w] bf16 AP; dst [128, NCI, *]
            fp8. If cast_fn is given, src is first filled from it (f32 ->
            bf16). Stats via ones-vector matmuls on the bf16 operand;
            per-token scale/shift broadcast via two K=1 matmuls per block;
            gamma/beta applied as ACT Identity with per-partition scale/bias.
            Phases are split across blocks so one block's PE stats overlap
            another block's row math."""
            with (
                nc.named_scope(scopename),
                tc.tile_pool(name=f"{scopename}_sb", bufs=max(3, n_blocks + 1)) as sbp,
            ):
                psp = psp1 = ps_all
                stats = []
                for tb in range(n_blocks):
                    sl = slice(tb * blk_w, (tb + 1) * blk_w)
                    psx = psp1.tile([1, blk_w], f32, tag="mm", name=f"psx{tb}")
                    psq = psp1.tile([1, blk_w], f32, tag="mm", name=f"psq{tb}")
                    if cast_fn is not None:
                        for ci in range(NCI):
                            nc.gpsimd.tensor_copy(
                                out=src_ap_fn(ci, sl), in_=cast_fn(ci, sl)
                            )
                    for ci in range(NCI):
                        nc.tensor.matmul(
                            psx, ones128, src_ap_fn(ci, sl),
                            start=(ci == 0), stop=(ci == NCI - 1),
                        )
                    for ci in range(NCI):
                        sq = sbp.tile([128, blk_w], bf16, tag="sq", name=f"sq{tb}_{ci}")
                        if ci % 2 == 0:
                            nc.scalar.activation(
                                out=sq, in_=src_ap_fn(ci, sl), func=Act.Square
                            )
                        else:
                            nc.vector.tensor_mul(
                                out=sq, in0=src_ap_fn(ci, sl), in1=src_ap_fn(ci, sl)
                            )
                        nc.tensor.matmul(
                            psq, ones128, sq,
                            start=(ci == 0), stop=(ci == NCI - 1),
                        )
                    stats.append((psx, psq))
                bcs = []
                for tb in range(n_blocks):
                    psx, psq = stats[tb]
                    mu = sbp.tile([1, blk_w], f32r, tag=f"r0_{tb}", bufs=1)
                    nc.scalar.mul(out=mu, in_=psx, mul=1.0 / C)
                    ms = sbp.tile([1, blk_w], f32r, tag=f"r1_{tb}", bufs=1)
                    nc.scalar.mul(out=ms, in_=psq, mul=1.0 / C)
                    mu2 = sbp.tile([1, blk_w], f32r, tag=f"r2_{tb}", bufs=1)
                    nc.scalar.activation(out=mu2, in_=mu, func=Act.Square)
                    var = sbp.tile([1, blk_w], f32r, tag=f"r3_{tb}", bufs=1)
                    nc.vector.tensor_sub(out=var, in0=ms, in1=mu2)
                    # rstd = exp(-0.5*ln(var+eps)): two fast ACT row ops
                    # (ACT Rsqrt is blocked for accuracy; DVE recip is slow)
                    sd = sbp.tile([1, blk_w], f32r, tag=f"r4_{tb}", bufs=1)
                    nc.scalar.activation(
                        out=sd, in_=var, func=Act.Ln, bias=epst
                    )
                    c0 = sbp.tile([1, blk_w], bf16, tag=f"r5_{tb}", bufs=1)
                    nc.scalar.activation(
                        out=c0, in_=sd, func=Act.Exp, scale=-0.5
                    )
                    nmu = sbp.tile([1, blk_w], f32r, tag=f"r6_{tb}", bufs=1)
                    nc.scalar.mul(out=nmu, in_=mu, mul=-1.0)
                    c1 = sbp.tile([1, blk_w], bf16, tag=f"r7_{tb}", bufs=1)
                    nc.vector.tensor_mul(out=c1, in0=nmu, in1=c0)
                    bc0 = psp.tile([128, blk_w], f32, tag="mm", name=f"bc0_{tb}")
                    bc1 = psp.tile([128, blk_w], f32, tag="mm", name=f"bc1_{tb}")
                    nc.tensor.matmul(bc0, ones_row, c0)
                    nc.tensor.matmul(bc1, ones_row, c1)
                    bcs.append((bc0, bc1))
                for tb in range(n_blocks):
                    sl = slice(tb * blk_w, (tb + 1) * blk_w)
                    bc0, bc1 = bcs[tb]
                    for ci in range(NCI):
                        x_ap = src_ap_fn(ci, sl)
                        tmp = sbp.tile([128, blk_w], f32, tag="tmp", name=f"t{tb}_{ci}")
                        nc.vector.tensor_mul(out=tmp, in0=x_ap, in1=bc0)
                        tmp2 = sbp.tile([128, blk_w], f32, tag="tmp2", name=f"t2_{tb}_{ci}")
                        nc.vector.tensor_add(out=tmp2, in0=tmp, in1=bc1)
                        # gamma/beta on GpSimd (idle here; ACT is the
                        # kernel-front bottleneck): (x_hat * g) + b
                        nc.gpsimd.tensor_scalar(
                            out=dst[:, ci, sl], in0=tmp2,
                            scalar1=gp[:, ci : ci + 1],
                            scalar2=bp[:, ci : ci + 1],
                            op0=Alu.mult, op1=Alu.add,
                        )

        # ---------------- LN1 over all T tokens ----------------
        a1, free_a1 = tc.tile([128, NCI, T], f8, name="a1", side="right")
        layernorm(lambda ci, sl: raw[:, ci, sl], a1, ln1gp, ln1bp, NTB, cfg.BW, "ln1")
        # residual-1 operand with bproj folded in (saves a pass in proj)
        for ci in range(NCI):
            nc.scalar.activation(
                out=rawq[:, ci, :], in_=raw[:, ci, 0:TQ],
                func=Act.Identity, bias=bpjt[:, ci : ci + 1],
            )
        free_raw()

        # ---------------- QKV (fp8 DoubleRow over C) ----------------
        qt, free_qt = tc.tile([128, NCI, TQ], f8, name="qt")
        kt, free_kt = tc.tile([128, NCI, T], f8, name="kt")
        # vt[p, kbp, half, h, d]; d==HD column holds WS for the fused
        # softmax-denominator row (folds the 1/WS of v's weight scale)
        vt, free_vt = tc.tile([128, NKP, 2, H, HD + 1], f8, name="vt")
        for kbp in range(NKP):
            for half in range(2):
                nc.vector.tensor_copy(
                    out=vt[:, kbp, half, :, HD : HD + 1],
                    in_=ws_f.unsqueeze(2),
                )

        with nc.named_scope("qkv"):
            wpool = wstream
            psp = ps_all
            # Q: out [C, TQ]
            for qb in range(cfg.NQB):
                qsl = slice(qb * QW, (qb + 1) * QW)
                pq = [psp.tile([128, QW], f32, tag="mm", name=f"pq{i}") for i in range(NCI)]
                for c2 in range(NC2):
                    wt = wpool.tile([128, 2, C], f8, tag="w")
                    nc.sync.dma_start(out=wt, in_=wq[c2])
                    for co in range(NCI):
                        nc.tensor.matmul(
                            pq[co],
                            wt[:, :, 128 * co : 128 * (co + 1)],
                            a1[:, 2 * c2 : 2 * c2 + 2, qsl],
                            start=(c2 == 0), stop=(c2 == NC2 - 1),
                            perf_mode=DR,
                        )
                for co in range(NCI):
                    nc.vector.tensor_copy(out=qt[:, co, qsl], in_=pq[co])
            # K: out [C, T], token blocks
            for tb in range(NTB):
                sl = slice(tb * cfg.BW, (tb + 1) * cfg.BW)
                pk = [psp.tile([128, cfg.BW], f32, tag="mm", name=f"pk{i}") for i in range(NCI)]
                for c2 in range(NC2):
                    wt = wpool.tile([128, 2, C], f8, tag="w")
                    nc.sync.dma_start(out=wt, in_=wk[c2])
                    for co in range(NCI):
                        nc.tensor.matmul(
                            pk[co],
                            wt[:, :, 128 * co : 128 * (co + 1)],
                            a1[:, 2 * c2 : 2 * c2 + 2, sl],
                            start=(c2 == 0), stop=(c2 == NC2 - 1),
                            perf_mode=DR,
                        )
                for co in range(NCI):
                    nc.vector.tensor_copy(out=kt[:, co, sl], in_=pk[co])
            # V weights resident in their own ring (reused by both hf phases)
            vw = min(C, 512)
            nhalf = C // vw  # <=512-wide chunks of the d_all dimension
            hpc = vw // HD  # heads per chunk
            wvt = []
            for c2 in range(NC2):
                wt = wpool.tile([128, 2, C], f8, tag="wv", bufs=NC2)
                nc.sync.dma_start(out=wt, in_=wv[c2])
                wvt.append(wt)

        def v_phase(hf):
            """V projection for head-chunk hf (heads hf*8..hf*8+7), emitted
            between attention head groups so its DR matmul stream keeps the
            PE busy while ACT chews through earlier heads' exps."""
            psp = ps_all
            for kbg in range(2):
                kbs = range(kbg * 4, kbg * 4 + 4)
                pv = {
                    kb: psp.tile([128, vw], f32, tag="mm", name=f"pv{kb}_{hf}")
                    for kb in kbs
                }
                for c2 in range(NC2):
                    for kb in kbs:
                        nc.tensor.matmul(
                            pv[kb],
                            a1[:, 2 * c2 : 2 * c2 + 2, 128 * kb : 128 * (kb + 1)],
                            wvt[c2][:, :, vw * hf : vw * (hf + 1)],
                            start=(c2 == 0), stop=(c2 == NC2 - 1),
                            perf_mode=DR,
                        )
                for kb in kbs:
                    kbp, half = kb % NKP, kb // NKP
                    if kb % 2 == 0:
                        nc.vector.tensor_copy(
                            out=vt[:, kbp, half, hpc * hf : hpc * (hf + 1), 0:HD],
                            in_=pv[kb].rearrange("p (h d) -> p h d", h=hpc),
                        )
                    else:
                        nc.scalar.copy(
                            out=vt[:, kbp, half, hpc * hf : hpc * (hf + 1), 0:HD],
                            in_=pv[kb].rearrange("p (h d) -> p h d", h=hpc),
                        )

        # ---------------- attention ----------------
        # att holds, per head, O^T rows 0..HD-1 (unnormalized) and the
        # softmax denominator in row 64. Scores/AV carry the WS^2 weight
        # scale; exp folds it away and the vt WS column folds v's WS.
        # Per head: denominator row is PE-broadcast to 64 partitions, DVE
        # reciprocal (multi-partition, fast), normalize into fp8 attn8,
        # then one DoubleRow matmul transposes the head pair to att2.
        att, free_att = tc.tile([65, H, TQ], bf16, name="att", side="right")
        attn8, free_attn8 = tc.tile([64, H, TQ], f8, name="attn8", side="right")
        att2 = qt  # packed normalized heads reuse qt's storage (WAR-safe)
        offs, PTW = cfg.pt_offs
        with (
            nc.named_scope("attn"),
            tc.tile_pool(name="at_pt", bufs=6, side="right") as ptp,
        ):
            pssc = psav = psbc = ps_all

            def attn_hp(hp):
                heads = (2 * hp, 2 * hp + 1)
                # pts[p, half, col]: packed exp-scores, both parity halves
                pts = [
                    ptp.tile([128, 2, PTW], f8, tag="pt", name=f"pt{h}")
                    for h in heads
                ]
                # score matmuls grouped so kbp 1 and 3 share one PSUM tile
                # (their pts regions are adjacent under kb_order): 3 exp ops
                # per (head, half) instead of 4
                for half in range(2):
                    for kbps in ([0], [1, 3], [2]):
                        pss = [
                            pssc.tile([128, 512], f32, tag="mm", name=f"sc{h}")
                            for h in heads
                        ]
                        co = 0
                        for kbp in kbps:
                            kb = half * NKP + kbp
                            s = cfg.s_kb(kb)
                            n = TQ - s
                            kbsl = slice(128 * kb, 128 * (kb + 1))
                            for idx, h in enumerate(heads):
                                po = idx * HD
                                nc.tensor.matmul(
                                    pss[idx][:, co : co + n],
                                    kt[po : po + HD, hp, kbsl],
                                    qt[po : po + HD, hp, s:TQ],
                                    skip_group_check=True,
                                )
                            co += n
                        pto = offs[kbps[0]]
                        for idx, h in enumerate(heads):
                            nc.scalar.activation(
                                out=pts[idx][:, half, pto : pto + co],
                                in_=pss[idx][:, 0:co],
                                func=Act.Exp, scale=scale * inv_ws2,
                            )
                        # causal mask: multiply each kbp's diagonal zone by
                        # 0/1; contiguous 2D ops split over GpSimd/DVE
                        for kbp in kbps:
                            kb = half * NKP + kbp
                            w = cfg.mask_w(kb)
                            for idx, h in enumerate(heads):
                                zone = pts[idx][
                                    :, half, offs[kbp] : offs[kbp] + w
                                ]
                                eng = (
                                    nc.gpsimd
                                    if (kbp + idx) % 2 == 0
                                    else nc.vector
                                )
                                eng.tensor_mul(
                                    out=zone, in0=zone, in1=mskt[:, kb, 0:w]
                                )
                for idx, h in enumerate(heads):
                    ps_o = psav.tile([65, TQ], f32, tag="mm", name=f"av{h}")
                    for kbp in range(NKP):
                        s = cfg.s_kb(kbp)
                        nc.tensor.matmul(
                            ps_o[:, s:TQ],
                            vt[:, kbp, :, h, :],
                            pts[idx][:, :, offs[kbp] : offs[kbp] + TQ - s],
                            start=(kbp == 0), stop=(kbp == NKP - 1),
                            skip_group_check=True,
                            perf_mode=DR,
                        )
                    nc.vector.tensor_copy(out=att[:, h, :], in_=ps_o)

            # interleave the two V head-chunks with the attention head
            # groups that consume them: the V DR streams keep the PE fed
            # while ACT runs the neighbouring heads' exps
            v_phase(0)
            for hp in range(H // 4):
                attn_hp(hp)
            v_phase(1)
            for hp in range(H // 4, H // 2):
                attn_hp(hp)
            # normalize all heads and pack pairs to 128 partitions; emitted
            # after the head loop so the bc/pack psum tiles sit at the end
            # of the shared-pool slot rotation. One K=1 matmul broadcasts
            # the denominator row; reciprocal_approx_fast (single custom
            # DVE op, ~18 correct bits) inverts it; DVE applies into fp8.
            for hp in range(H // 2):
                heads = (2 * hp, 2 * hp + 1)
                for idx, h in enumerate(heads):
                    bc = psbc.tile([64, TQ], f32, tag="mm", name=f"bc{h}")
                    nc.tensor.matmul(bc, oneshi[64:65, :], att[64:65, h, :])
                    rc = ptp.tile([64, TQ], f32, tag="rc", name=f"rc{h}")
                    nc.vector.reciprocal_approx_fast(out=rc, in_=bc)
                    nc.vector.tensor_mul(
                        out=attn8[:, h, :],
                        in0=att[0:64, h, :],
                        in1=rc,
                    )
                # transpose the normalized pair into [128, TQ] via one
                # DoubleRow matmul against the packed fp8 identity
                pk = psbc.tile([128, TQ], f32, tag="mm", name=f"pk{hp}")
                nc.tensor.matmul(
                    pk,
                    identsb,
                    attn8[:, 2 * hp : 2 * hp + 2, :],
                    perf_mode=DR,
                )
                nc.vector.tensor_copy(out=att2[:, hp, :], in_=pk)

        # ---------------- attention out-proj + residual 1 ----------------
        with nc.named_scope("proj"):
            wpool = wstream
            psp = ps_all
            for qb in range(cfg.NQB):
                qsl = slice(qb * QW, (qb + 1) * QW)
                pp = [psp.tile([128, QW], f32, tag="mm", name=f"pp{i}") for i in range(NCI)]
                for c2 in range(NC2):
                    wt = wpool.tile([128, 2, C], f8, tag="w")
                    nc.sync.dma_start(out=wt, in_=wp[c2])
                    for co in range(NCI):
                        nc.tensor.matmul(
                            pp[co],
                            wt[:, :, 128 * co : 128 * (co + 1)],
                            att2[:, 2 * c2 : 2 * c2 + 2, qsl],
                            start=(c2 == 0), stop=(c2 == NC2 - 1),
                            perf_mode=DR,
                        )
                for co in range(NCI):
                    nc.vector.scalar_tensor_tensor(
                        out=x2t[:, co, qsl],
                        in0=pp[co],
                        scalar=inv64[:, 0:1],
                        in1=rawq[:, co, qsl],
                        op0=Alu.mult,
                        op1=Alu.add,
                    )
        free_attn8()
        free_att()
        free_a1()
        free_vt()
        free_kt()
        free_qt()
        free_mskt()
        free_rawq()
        yts, free_yts = tc.tile([128, NCI, TQ], f32, name="yts")
        # x2 + b2 precomputed so ffn2's epilogue is a single stt op
        x2b, free_x2b = tc.tile([128, NCI, TQ], f32, name="x2b")
        for ci in range(NCI):
            nc.scalar.activation(
                out=x2b[:, ci, :], in_=x2t[:, ci, :],
                func=Act.Identity, bias=b2t[:, ci : ci + 1],
            )

        # ---------------- LN2 ----------------
        a2, free_a2 = tc.tile([128, NCI, TQ], bf16, name="a2", side="right")
        x2c, free_x2c = tc.tile([128, NCI, TQ], bf16, name="x2c", side="right")
        layernorm(
            lambda ci, sl: x2c[:, ci, sl], a2, ln2gp, ln2bp, cfg.NQB, QW,
            "ln2", cast_fn=lambda ci, sl: x2t[:, ci, sl],
        )

        # ---------------- FFN (bf16: fp8 here costs too much accuracy) ----
        hsb, free_hsb = tc.tile([128, NFF, QW], bf16, name="hsb", side="right")
        with nc.named_scope("ffn1"):
            wpool = wstream
            psp = ps_all
            for qb in range(cfg.NQB):
                qsl = slice(qb * QW, (qb + 1) * QW)
                for cog in range(NFF // 8):
                    pf = [psp.tile([128, QW], f32, tag="mm", name=f"pf{i}") for i in range(8)]
                    for ci in range(NCI):
                        wt = wpool.tile([128, 1024], bf16, tag="w")
                        nc.sync.dma_start(
                            out=wt,
                            in_=w1[
                                128 * ci : 128 * (ci + 1),
                                1024 * cog : 1024 * (cog + 1),
                            ],
                        )
                        for co in range(8):
                            nc.tensor.matmul(
                                pf[co],
                                wt[:, 128 * co : 128 * (co + 1)],
                                a2[:, ci, qsl],
                                start=(ci == 0), stop=(ci == NCI - 1),
                            )
                    for co in range(8):
                        hco = cog * 8 + co
                        nc.scalar.activation(
                            out=hsb[:, hco, qsl],
                            in_=pf[co],
                            func=Act.Gelu,
                            bias=b1t[:, hco : hco + 1],
                        )

        with nc.named_scope("ffn2"):
            wpool = wstream
            psp = ps_all
            for qb in range(cfg.NQB):
                qsl = slice(qb * QW, (qb + 1) * QW)
                py = [psp.tile([128, QW], f32, tag="mm", name=f"py{i}") for i in range(NCI)]
                for fi in range(NFF):
                    wt = wpool.tile([128, C], bf16, tag="w")
                    nc.sync.dma_start(out=wt, in_=w2[128 * fi : 128 * (fi + 1)])
                    for co in range(NCI):
                        nc.tensor.matmul(
                            py[co],
                            wt[:, 128 * co : 128 * (co + 1)],
                            hsb[:, fi, qsl],
                            start=(fi == 0), stop=(fi == NFF - 1),
                        )
                for co in range(NCI):
                    # yts = py + (x2 + b2)
                    nc.vector.tensor_add(
                        out=yts[:, co, qsl],
                        in0=py[co],
                        in1=x2b[:, co, qsl],
                    )
        nc.sync.dma_start(
            out=yt.rearrange("(ci p) t -> p ci t", p=128), in_=yts
        )
        free_hsb()
        free_x2c()
        free_a2()
        free_x2b()
        free_yts()
        free_x2t()
        wstream.release()
        ps_all.release()
        free_b2t()
        free_b1t()
        free_bpjt()
        free_ln2bp()
        free_ln2gp()
        free_ln1bp()
        free_ln1gp()
        free_identsb()
        free_epst()
        free_oneshi()
        free_ones_row()
        free_ones128()
        free_inv64()
        free_ws_f()
        free_onesf()

    nc.compile()
    return nc


def prep_core_inputs(cfg: Cfg, inputs: dict, b: int, j: int) -> dict:
    """Host-side slicing/permutation for core (batch b, parity j)."""
    T, TQ, NKB, MW = cfg.T, cfg.TQ, cfg.NKB, cfg.MW
    x = np.asarray(inputs["x"])
    perm = np.concatenate([np.arange(j, T, 2), np.arange(1 - j, T, 2)])
    xp = x[b][perm]  # [T, C]
    xpt = np.ascontiguousarray(xp.T, dtype=np.float32)

    import ml_dtypes

    qtok = perm[:TQ]
    ktok = perm
    mask = np.ones((NKB, 128, MW), dtype=np.float32)
    for kb in range(NKB):
        s = cfg.s_kb(kb)
        w = cfg.mask_w(kb)
        kt = ktok[128 * kb : 128 * (kb + 1)]  # [128]
        qt = qtok[s : s + w]  # [w]
        allowed = qt[None, :] >= kt[:, None]  # [128, w]
        mask[kb, :, :w] = np.where(allowed, 1.0, 0.0)
    return {
        "xpt": xpt.astype(ml_dtypes.bfloat16),
        "msk": mask.astype(ml_dtypes.float8_e4m3fn),
    }


def _pack_dr(w: np.ndarray) -> np.ndarray:
    """[K, F] f32 -> [K/256, 128, 2, F] fp8 with the WS scale applied."""
    import ml_dtypes

    K, F = w.shape
    wp = (w * WS).reshape(K // 256, 2, 128, F).transpose(0, 2, 1, 3)
    return np.ascontiguousarray(wp.astype(ml_dtypes.float8_e4m3fn))


def prep_shared_inputs(cfg: Cfg, inputs: dict) -> dict:
    import ml_dtypes

    C = cfg.C
    bf16 = ml_dtypes.bfloat16

    def wq2d(w):  # [H, C, HD] -> [C, H*HD]
        w = np.asarray(w, dtype=np.float32)
        return np.ascontiguousarray(w.transpose(1, 0, 2).reshape(C, C))

    HD = cfg.HD
    ident = np.zeros((HD, 2, 128), dtype=ml_dtypes.float8_e4m3fn)
    ident[np.arange(HD), 0, np.arange(HD)] = 1.0
    ident[np.arange(HD), 1, HD + np.arange(HD)] = 1.0
    f32 = np.float32
    return {
        "ident": ident,
        "wq": _pack_dr(wq2d(inputs["Wq"])),
        "wk": _pack_dr(wq2d(inputs["Wk"])),
        "wv": _pack_dr(wq2d(inputs["Wv"])),
        "wp": _pack_dr(np.asarray(inputs["Wproj"], dtype=f32)),
        "w1": np.ascontiguousarray(np.asarray(inputs["W1"]).astype(bf16)),
        "w2": np.ascontiguousarray(np.asarray(inputs["W2"]).astype(bf16)),
        "ln1g": np.ascontiguousarray(inputs["ln1_g"], dtype=f32),
        "ln1b": np.ascontiguousarray(inputs["ln1_b"], dtype=f32),
        "ln2g": np.ascontiguousarray(inputs["ln2_g"], dtype=f32),
        "ln2b": np.ascontiguousarray(inputs["ln2_b"], dtype=f32),
        "bpj": np.ascontiguousarray(inputs["bproj"], dtype=f32),
        "b1": np.ascontiguousarray(inputs["b1"], dtype=f32),
        "b2": np.ascontiguousarray(inputs["b2"], dtype=f32),
    }


def run(cfg: Cfg, inputs: dict, n_cores: int = 8, trace: bool = False):
    from concourse.bass_utils import run_bass_kernel_spmd

    nc = build_nc(cfg, n_cores=n_cores)
    shared = prep_shared_inputs(cfg, inputs)
    in_maps = []
    cores = []
    for core in range(n_cores):
        b, j = divmod(core, 2)
        b = b % cfg.B
        in_maps.append({**prep_core_inputs(cfg, inputs, b, j), **shared})
        cores.append((b, j))
    res = run_bass_kernel_spmd(
        nc, in_maps, core_ids=list(range(n_cores)), trace=trace
    )
    out = np.zeros((cfg.B, cfg.T, cfg.C), dtype=np.float32)
    for core, (b, j) in enumerate(cores):
        ytv = res.results[core]["yt"]  # [C, TQ]
        perm = np.concatenate(
            [np.arange(j, cfg.T, 2), np.arange(1 - j, cfg.T, 2)]
        )
        out[b, perm[: cfg.TQ], :] = ytv.T
    return out, res


def kernel(**inputs) -> np.ndarray:
    out, _ = run(Cfg(), inputs, n_cores=8, trace=False)
    return out


if __name__ == "__main__":
    # quick self-exercise at full size with random data
    rng = np.random.default_rng(0)
    cfg = Cfg()
    ins = {
        "x": rng.standard_normal((cfg.B, cfg.T, cfg.C)).astype(np.float32),
        "ln1_g": np.ones(cfg.C, np.float32),
        "ln1_b": np.zeros(cfg.C, np.float32),
        "ln2_g": np.ones(cfg.C, np.float32),
        "ln2_b": np.zeros(cfg.C, np.float32),
        "Wq": rng.standard_normal((cfg.H, cfg.C, cfg.HD)).astype(np.float32)
        * 0.02,
        "Wk": rng.standard_normal((cfg.H, cfg.C, cfg.HD)).astype(np.float32)
        * 0.02,
        "Wv": rng.standard_normal((cfg.H, cfg.C, cfg.HD)).astype(np.float32)
        * 0.02,
        "Wproj": rng.standard_normal((cfg.C, cfg.C)).astype(np.float32) * 0.02,
        "bproj": np.zeros(cfg.C, np.float32),
        "W1": rng.standard_normal((cfg.C, cfg.FF)).astype(np.float32) * 0.02,
        "b1": np.zeros(cfg.FF, np.float32),
        "W2": rng.standard_normal((cfg.FF, cfg.C)).astype(np.float32) * 0.02,
        "b2": np.zeros(cfg.C, np.float32),
    }
    y = kernel(**ins)
    print("ran, out", y.shape, y.dtype, float(np.abs(y).max()))
